# revision 36
# baseline (speedup 1.0000x reference)
"""Trainium2 Bass kernel for nn_AlignModel.

Computes out[b, j, i] = sigmoid(simp[b,j]·w_s + orig[b,i]·w_o + bias) where
orig/simp are the two halves of prop_state[b] ([B, 2S, D] -> [B,S,D] each),
w_o = W[0,:D], w_s = W[0,D:].

Sharding: data-parallel over batch B=8 across the 8 NeuronCores. Each core:
  in  x   [4096, 512] f32, out [2048, 2048] fp16 (host upcasts to f32;
  sigmoid outputs with logits within +-5 keep rel-err ~1e-3 in fp16),
  so per-core HBM traffic is 8 MiB in + 8 MiB out.

v4a schedule (every choice below is measured-HW-driven):
  - w arrives host-replicated [128, 2D] (one fast contiguous DMA per
    half; the on-device zero-stride broadcast was pathological), b as a
    host-built fp16 [1, 512] row.
  - orig is partition-OUTER (i = p*16+n) in chunks of 2-4 n-slices:
    4-8 KiB descriptors run each HWDGE queue ~2x faster than the 2 KiB
    ones partition-inner forces (~190 -> ~300+ GB/s per queue).
  - simp stays partition-inner (the sigmoid bias needs [128,1] columns)
    and is WAW-gated to release mid-phase-1a.
  - s_o row formation, all on idle engines at end of phase 1a:
    PE transpose -> DVE fp16 cast (tr16[n,p] = s_o[p*16+n]) -> GpSimd
    masked expansion row_sp[n, (p,ns)] = tr16[n,p] * (n==ns) via a
    zero-stride broadcast view -> 4 all-ones rank-16 MMs accumulate the
    diagonal onto the b seed in sob PSUM.
  - All 32 dots are fused DVE scalar_tensor_tensor; ScalarE runs only
    the 16 sigmoids: out_t = Sigmoid(sob + bias_col_t), PSUM->SBUF fp16.
  - Stores: tile 0 single + pairs on Sync; tile 15 via GpSimd SWDGE as
    a store-bandwidth probe.
"""

import numpy as np

import concourse.mybir as mybir
from concourse import bacc, bass_utils
from concourse.masks import make_identity
from concourse.tile import TileContext

P = 128          # partitions
D = 512          # feature dim
S = 2048         # sents
NT = S // P      # 16 tiles per half
OCHUNKS = [2, 4, 4, 3, 3]      # orig tiles per chunk (p-inner fp16)
OQ = ["sync", "scalar", "sync", "scalar", "sync"]
SCH = 4          # simp tiles per group (1 MiB)
NSC = NT // SCH
SQ = ["sync", "scalar", "sync", "scalar"]
GATE_AT = {6: 0, 8: 1, 10: 2, 12: 3}   # orig dot t -> simp group released
NCORES = 8
F32 = mybir.dt.float32
F16 = mybir.dt.float16


def _kernel_body(tc, out, x, w, bvec):
    nc = tc.nc
    # BOTH halves partition-inner: row = n*P + p
    xo_re = x[0:S, :].rearrange("(n p) d -> p n d", p=P)
    xs_re = x[S:2 * S, :].rearrange("(n p) d -> p n d", p=P)
    eng = {"sync": nc.sync, "scalar": nc.scalar}

    with (
        tc.tile_pool(name="consts", bufs=1) as cpool,
        tc.tile_pool(name="xin", bufs=1) as xpool,
        tc.tile_pool(name="scratch", bufs=4) as spool,
        tc.tile_pool(name="outbuf", bufs=4) as opool,
        tc.tile_pool(name="psum", bufs=1, space="PSUM") as ppool,
        tc.tile_pool(name="trpsum", bufs=2, space="PSUM") as tpool,
    ):
        w_bc = cpool.tile([P, 2 * D], F16, tag="wbc")
        nc.scalar.dma_start(out=w_bc[:, 0:D], in_=w[0])
        nc.scalar.dma_start(out=w_bc[:, D:2 * D], in_=w[1])
        b_row = cpool.tile([1, 512], F16, tag="brow")
        nc.gpsimd.dma_start(out=b_row, in_=bvec)

        # --- orig input stream: chunks alternate between the two queues ---
        xo_tiles = []
        n0 = 0
        for c, sz in enumerate(OCHUNKS):
            xo = xpool.tile([P, sz, D], F16, tag=f"xo{c}", name=f"xo{c}")
            eng[OQ[c]].dma_start(out=xo, in_=xo_re[:, n0:n0 + sz, :])
            xo_tiles.append(xo)
            n0 += sz

        xs_tiles = [
            xpool.tile([P, SCH, D], F16, tag=f"xs{g}", name=f"xs{g}")
            for g in range(NSC)
        ]

        ones_row = cpool.tile([1, P], F16, tag="ones")
        nc.gpsimd.memset(ones_row, 1.0)
        # preload the Sigmoid table while ScalarE is idle (the implicit
        # ACT_TABLE_LOAD otherwise lands right before the first sigmoid)
        warm = cpool.tile([1, 2], F32, tag="warm")
        nc.gpsimd.memset(warm, 0.0)
        nc.scalar.activation(warm[:, 1:2], warm[:, 0:1],
                             mybir.ActivationFunctionType.Sigmoid)
        ident = cpool.tile([P, P], F32, tag="ident")
        make_identity(nc, ident)
        # sel_big[0:sz, m*P:(m+1)*P] one-hot-selects local row m of a
        # transposed chunk into sob's free block (PE operands must sit at
        # base partition 0, so the block choice lives in the stationary).
        sel_big = cpool.tile([NT, S], F16, tag="selbig")
        nc.gpsimd.memset(sel_big, 0.0)
        sel3 = sel_big.rearrange("m (n j) -> m n j", j=P)
        nc.gpsimd.affine_select(
            out=sel3, in_=sel3, compare_op=mybir.AluOpType.not_equal,
            fill=1.0, base=0, pattern=[[-1, NT], [0, P]],
            channel_multiplier=1)

        s_o_mat = cpool.tile([P, NT], F32, tag="somat")   # s_o[p*16+n] @ [p,n]
        s_sb_mat = cpool.tile([P, NT], F32, tag="ssmat")  # s_s col t
        sob_psum = ppool.tile([P, S], F32, tag="sob")     # b + s_o, every row

        # bias seeds (128-col blocks, matching the accumulation MMs below)
        for n in range(NT):
            nc.tensor.matmul(sob_psum[:, n * P:(n + 1) * P], ones_row,
                             b_row[:, 0:P], start=True, stop=False,
                             skip_group_check=True)

        # --- phase 1a: orig dots ---
        n0 = 0
        for c, sz in enumerate(OCHUNKS):
            xo = xo_tiles[c]
            for blk in range(sz):
                t = n0 + blk
                prod = spool.tile([P, D], F16, tag="prod", name=f"po{t}")
                nc.vector.scalar_tensor_tensor(
                    out=prod, in0=xo[:, blk, :], scalar=1.0,
                    in1=w_bc[:, 0:D],
                    op0=mybir.AluOpType.mult, op1=mybir.AluOpType.mult,
                    accum_out=s_o_mat[:, t:t + 1])
                if t in GATE_AT:
                    # WAW gate: emitted BEFORE the simp dma_start below so
                    # the load waits for this write.
                    g = GATE_AT[t]
                    nc.vector.tensor_copy(
                        out=xs_tiles[g][0:1, 0, 0:1],
                        in_=prod[0:1, 0:1])
            # per-chunk: transpose the s_o columns into contiguous row
            # slices (partition-inner!), cast to fp16, route each local
            # row into its sob block via a one-hot stationary MM.
            tr = tpool.tile([sz, P], F32, tag="tr", name=f"tr{c}")
            nc.tensor.transpose(tr, s_o_mat[:, n0:n0 + sz], ident)
            so16_c = spool.tile([sz, P], F16, tag="so16", name=f"so16c{c}")
            nc.vector.tensor_copy(out=so16_c, in_=tr)
            for m in range(sz):
                n = n0 + m
                nc.tensor.matmul(sob_psum[:, n * P:(n + 1) * P],
                                 sel_big[0:sz, m * P:(m + 1) * P], so16_c,
                                 start=False, stop=True,
                                 skip_group_check=True)
            n0 += sz

        # simp loads: emitted after the gate writes (WAW order)
        for g in range(NSC):
            eng[SQ[g]].dma_start(out=xs_tiles[g],
                                 in_=xs_re[:, g * SCH:(g + 1) * SCH, :])

        # --- phase 1b + 2: simp dots, then the sigmoid stream ---
        o_sb = None
        for g in range(NSC):
            xs = xs_tiles[g]
            for blk in range(SCH):
                t = g * SCH + blk
                prod = spool.tile([P, D], F16, tag="prod", name=f"ps{t}")
                nc.vector.scalar_tensor_tensor(
                    out=prod, in0=xs[:, blk, :], scalar=1.0,
                    in1=w_bc[:, D:2 * D],
                    op0=mybir.AluOpType.mult, op1=mybir.AluOpType.mult,
                    accum_out=s_sb_mat[:, t:t + 1])
            for blk in range(SCH):
                t = g * SCH + blk
                if t in (0, NT - 1):
                    o_sb = opool.tile([P, 2, S], F16, tag="osb",
                                      name=f"osingle{t}")
                    nc.scalar.activation(
                        o_sb[:, 0, :], sob_psum,
                        mybir.ActivationFunctionType.Sigmoid,
                        bias=s_sb_mat[:, t:t + 1], scale=1.0)
                    nc.sync.dma_start(out=out[t * P:(t + 1) * P, :],
                                      in_=o_sb[:, 0, :])
                    continue
                q = (t - 1) % 2
                if q == 0:
                    o_sb = opool.tile([P, 2, S], F16, tag="osb",
                                      name=f"opair{t // 2}")
                nc.scalar.activation(
                    o_sb[:, q, :], sob_psum,
                    mybir.ActivationFunctionType.Sigmoid,
                    bias=s_sb_mat[:, t:t + 1],
                    scale=1.0,
                )
                if q == 1:
                    r0 = (t - 1) * P
                    dst = out[r0:r0 + 2 * P, :].rearrange(
                        "(q p) i -> p q i", p=P)
                    nc.sync.dma_start(out=dst, in_=o_sb)


def build_program():
    nc = bacc.Bacc(
        "TRN2",
        debug=False,
        target_bir_lowering=False,
        num_devices=NCORES,
    )
    x = nc.dram_tensor("x", [2 * S, D], F16, kind="ExternalInput").ap()
    w = nc.dram_tensor("w", [2, P, D], F16, kind="ExternalInput").ap()
    bvec = nc.dram_tensor("bvec", [1, 512], F16, kind="ExternalInput").ap()
    out = nc.dram_tensor("out", [S, S], F16, kind="ExternalOutput").ap()
    with TileContext(nc) as tc:
        _kernel_body(tc, out, x, w, bvec)
    nc.compile()
    return nc


_PROGRAM = None


def _get_program():
    global _PROGRAM
    if _PROGRAM is None:
        _PROGRAM = build_program()
    return _PROGRAM


def make_in_maps(prop_state, W, b):
    prop = np.ascontiguousarray(np.asarray(prop_state).astype(np.float16))
    w2 = np.asarray(W, dtype=np.float32).reshape(2, 1, D).astype(np.float16)
    w = np.ascontiguousarray(np.broadcast_to(w2, (2, P, D)))
    bv = np.ascontiguousarray(np.broadcast_to(
        np.asarray(b, dtype=np.float16).reshape(1, 1), (1, 512)))
    assert prop.shape == (NCORES, 2 * S, D), prop.shape
    return [{"x": prop[i], "w": w, "bvec": bv} for i in range(NCORES)]


def kernel(A, prop_state, W, b, _trace=False):
    nc = _get_program()
    in_maps = make_in_maps(prop_state, W, b)
    res = bass_utils.run_bass_kernel_spmd(
        nc, in_maps, core_ids=list(range(NCORES)), trace=_trace)
    out = np.stack(
        [res.results[i]["out"] for i in range(NCORES)], axis=0
    ).astype(np.float32)
    if _trace:
        kernel.last_results = res
    return out


# revision 37
# speedup vs baseline: 1.1052x; 1.1052x over previous
"""Trainium2 Bass kernel for nn_AlignModel.

Computes out[b, j, i] = sigmoid(simp[b,j]·w_s + orig[b,i]·w_o + bias) where
orig/simp are the two halves of prop_state[b] ([B, 2S, D] -> [B,S,D] each),
w_o = W[0,:D], w_s = W[0,D:].

Sharding: data-parallel over batch B=8 across the 8 NeuronCores. Each core:
  in  x   [4096, 512] f32, out [2048, 2048] fp16 (host upcasts to f32;
  sigmoid outputs with logits within +-5 keep rel-err ~1e-3 in fp16),
  so per-core HBM traffic is 8 MiB in + 8 MiB out.

v4a schedule (every choice below is measured-HW-driven):
  - w arrives host-replicated [128, 2D] (one fast contiguous DMA per
    half; the on-device zero-stride broadcast was pathological), b as a
    host-built fp16 [1, 512] row.
  - orig is partition-OUTER (i = p*16+n) in chunks of 2-4 n-slices:
    4-8 KiB descriptors run each HWDGE queue ~2x faster than the 2 KiB
    ones partition-inner forces (~190 -> ~300+ GB/s per queue).
  - simp stays partition-inner (the sigmoid bias needs [128,1] columns)
    and is WAW-gated to release mid-phase-1a.
  - s_o row formation, all on idle engines at end of phase 1a:
    PE transpose -> DVE fp16 cast (tr16[n,p] = s_o[p*16+n]) -> GpSimd
    masked expansion row_sp[n, (p,ns)] = tr16[n,p] * (n==ns) via a
    zero-stride broadcast view -> 4 all-ones rank-16 MMs accumulate the
    diagonal onto the b seed in sob PSUM.
  - All 32 dots are fused DVE scalar_tensor_tensor; ScalarE runs only
    the 16 sigmoids: out_t = Sigmoid(sob + bias_col_t), PSUM->SBUF fp16.
  - Stores: tile 0 single + pairs on Sync; tile 15 via GpSimd SWDGE as
    a store-bandwidth probe.
"""

import numpy as np

import concourse.mybir as mybir
from concourse import bacc, bass_utils
from concourse.masks import make_identity
from concourse.tile import TileContext

P = 128          # partitions
D = 512          # feature dim
S = 2048         # sents
NT = S // P      # 16 tiles per half
OCHUNKS = [2, 5, 4, 3, 2]      # orig n-slices (fp16, desc 2-5KiB)
OQ = ["sync", "scalar", "sync", "scalar", "gpsimd"]
SCH = 4          # simp tiles per group (1 MiB)
NSC = NT // SCH
SQ = ["sync", "scalar", "sync", "scalar"]
GATE_AT = {6: 0, 8: 1, 10: 2, 12: 3}   # orig dot t -> simp group released
NCORES = 8
F32 = mybir.dt.float32
F16 = mybir.dt.float16


def _kernel_body(tc, out, x, w, bvec):
    nc = tc.nc
    # orig partition-outer: i = p*NT + n ; simp partition-inner: j = n*P + p
    xo_re = x[0:S, :].rearrange("(p n) d -> p n d", n=NT)
    xs_re = x[S:2 * S, :].rearrange("(n p) d -> p n d", p=P)
    eng = {"sync": nc.sync, "scalar": nc.scalar, "gpsimd": nc.gpsimd}

    with (
        tc.tile_pool(name="consts", bufs=1) as cpool,
        tc.tile_pool(name="xin", bufs=1) as xpool,
        tc.tile_pool(name="scratch", bufs=4) as spool,
        tc.tile_pool(name="outbuf", bufs=4) as opool,
        tc.tile_pool(name="psum", bufs=1, space="PSUM") as ppool,
        tc.tile_pool(name="trpsum", bufs=2, space="PSUM") as tpool,
    ):
        w_bc = cpool.tile([P, 2 * D], F16, tag="wbc")
        nc.scalar.dma_start(out=w_bc[:, 0:D], in_=w[0])
        nc.scalar.dma_start(out=w_bc[:, D:2 * D], in_=w[1])
        b_row = cpool.tile([1, 512], F16, tag="brow")
        nc.gpsimd.dma_start(out=b_row, in_=bvec)

        # --- orig input stream: chunks alternate between the two queues ---
        xo_tiles = []
        n0 = 0
        for c, sz in enumerate(OCHUNKS):
            xo = xpool.tile([P, sz, D], F16, tag=f"xo{c}", name=f"xo{c}")
            eng[OQ[c]].dma_start(out=xo, in_=xo_re[:, n0:n0 + sz, :])
            xo_tiles.append(xo)
            n0 += sz

        xs_tiles = [
            xpool.tile([P, SCH, D], F16, tag=f"xs{g}", name=f"xs{g}")
            for g in range(NSC)
        ]

        ones_row = cpool.tile([1, P], F16, tag="ones")
        nc.gpsimd.memset(ones_row, 1.0)
        # preload the Sigmoid table while ScalarE is idle (the implicit
        # ACT_TABLE_LOAD otherwise lands right before the first sigmoid)
        warm = cpool.tile([1, 2], F32, tag="warm")
        nc.gpsimd.memset(warm, 0.0)
        nc.scalar.activation(warm[:, 1:2], warm[:, 0:1],
                             mybir.ActivationFunctionType.Sigmoid)
        ident = cpool.tile([P, P], F32, tag="ident")
        make_identity(nc, ident)
        ones16 = cpool.tile([NT, P], F16, tag="ones16")
        nc.gpsimd.memset(ones16, 1.0)
        # mask16[m, (p, ns)] = (m == ns): the diagonal selector that turns
        # the partition-broadcast of tr16 into the interleaved s_o row.
        mask16 = cpool.tile([NT, S], F16, tag="mask16")
        nc.gpsimd.memset(mask16, 0.0)
        mask3 = mask16.rearrange("m (p ns) -> m p ns", ns=NT)
        nc.gpsimd.affine_select(
            out=mask3, in_=mask3, compare_op=mybir.AluOpType.not_equal,
            fill=1.0, base=0, pattern=[[0, P], [-1, NT]],
            channel_multiplier=1)

        s_o_mat = cpool.tile([P, NT], F32, tag="somat")   # s_o[p*16+n] @ [p,n]
        s_sb_mat = cpool.tile([P, NT], F32, tag="ssmat")  # s_s col t
        row_sp = cpool.tile([NT, S], F16, tag="rowsp")
        sob_psum = ppool.tile([P, S], F32, tag="sob")     # b + s_o, every row

        # bias seeds (FD=512 blocks, matching the accumulation MMs below)
        for jj in range(S // 512):
            nc.tensor.matmul(sob_psum[:, jj * 512:(jj + 1) * 512], ones_row,
                             b_row, start=True, stop=False,
                             skip_group_check=True)

        # --- phase 1a: orig dots ---
        n0 = 0
        for c, sz in enumerate(OCHUNKS):
            xo = xo_tiles[c]
            for blk in range(sz):
                t = n0 + blk
                prod = spool.tile([P, D], F16, tag="prod", name=f"po{t}")
                nc.vector.scalar_tensor_tensor(
                    out=prod, in0=xo[:, blk, :], scalar=1.0,
                    in1=w_bc[:, 0:D],
                    op0=mybir.AluOpType.mult, op1=mybir.AluOpType.mult,
                    accum_out=s_o_mat[:, t:t + 1])
                if t in GATE_AT:
                    # WAW gate: emitted BEFORE the simp dma_start below so
                    # the load waits for this write.
                    g = GATE_AT[t]
                    nc.vector.tensor_copy(
                        out=xs_tiles[g][0:1, 0, 0:1],
                        in_=prod[0:1, 0:1])
            n0 += sz

        # s_o -> sob: transpose, cast, masked diagonal expansion (GpSimd,
        # zero-stride broadcast read), 4 all-ones rank-16 MMs.
        tr = tpool.tile([NT, P], F32, tag="tr", name="trso")
        nc.tensor.transpose(tr, s_o_mat, ident)
        tr_bc = tr.unsqueeze(2).broadcast_to([NT, P, NT])
        nc.vector.tensor_tensor(
            out=row_sp.rearrange("m (p ns) -> m p ns", ns=NT),
            in0=tr_bc, in1=mask3, op=mybir.AluOpType.mult)
        for jj in range(S // 512):
            nc.tensor.matmul(sob_psum[:, jj * 512:(jj + 1) * 512], ones16,
                             row_sp[:, jj * 512:(jj + 1) * 512],
                             start=False, stop=True, skip_group_check=True)

        # simp loads: emitted after the gate writes (WAW order)
        for g in range(NSC):
            eng[SQ[g]].dma_start(out=xs_tiles[g],
                                 in_=xs_re[:, g * SCH:(g + 1) * SCH, :])

        # --- phase 1b + 2: simp dots, then the sigmoid stream ---
        o_sb = None
        for g in range(NSC):
            xs = xs_tiles[g]
            for blk in range(SCH):
                t = g * SCH + blk
                prod = spool.tile([P, D], F16, tag="prod", name=f"ps{t}")
                nc.vector.scalar_tensor_tensor(
                    out=prod, in0=xs[:, blk, :], scalar=1.0,
                    in1=w_bc[:, D:2 * D],
                    op0=mybir.AluOpType.mult, op1=mybir.AluOpType.mult,
                    accum_out=s_sb_mat[:, t:t + 1])
            for blk in range(SCH):
                t = g * SCH + blk
                if t in (0, NT - 1):
                    o_sb = opool.tile([P, 2, S], F16, tag="osb",
                                      name=f"osingle{t}")
                    nc.scalar.activation(
                        o_sb[:, 0, :], sob_psum,
                        mybir.ActivationFunctionType.Sigmoid,
                        bias=s_sb_mat[:, t:t + 1], scale=1.0)
                    nc.sync.dma_start(out=out[t * P:(t + 1) * P, :],
                                      in_=o_sb[:, 0, :])
                    continue
                q = (t - 1) % 2
                if q == 0:
                    o_sb = opool.tile([P, 2, S], F16, tag="osb",
                                      name=f"opair{t // 2}")
                nc.scalar.activation(
                    o_sb[:, q, :], sob_psum,
                    mybir.ActivationFunctionType.Sigmoid,
                    bias=s_sb_mat[:, t:t + 1],
                    scale=1.0,
                )
                if q == 1:
                    r0 = (t - 1) * P
                    dst = out[r0:r0 + 2 * P, :].rearrange(
                        "(q p) i -> p q i", p=P)
                    nc.sync.dma_start(out=dst, in_=o_sb)


def build_program():
    nc = bacc.Bacc(
        "TRN2",
        debug=False,
        target_bir_lowering=False,
        num_devices=NCORES,
    )
    x = nc.dram_tensor("x", [2 * S, D], F16, kind="ExternalInput").ap()
    w = nc.dram_tensor("w", [2, P, D], F16, kind="ExternalInput").ap()
    bvec = nc.dram_tensor("bvec", [1, 512], F16, kind="ExternalInput").ap()
    out = nc.dram_tensor("out", [S, S], F16, kind="ExternalOutput").ap()
    with TileContext(nc) as tc:
        _kernel_body(tc, out, x, w, bvec)
    nc.compile()
    return nc


_PROGRAM = None


def _get_program():
    global _PROGRAM
    if _PROGRAM is None:
        _PROGRAM = build_program()
    return _PROGRAM


def make_in_maps(prop_state, W, b):
    prop = np.ascontiguousarray(np.asarray(prop_state).astype(np.float16))
    w2 = np.asarray(W, dtype=np.float32).reshape(2, 1, D).astype(np.float16)
    w = np.ascontiguousarray(np.broadcast_to(w2, (2, P, D)))
    bv = np.ascontiguousarray(np.broadcast_to(
        np.asarray(b, dtype=np.float16).reshape(1, 1), (1, 512)))
    assert prop.shape == (NCORES, 2 * S, D), prop.shape
    return [{"x": prop[i], "w": w, "bvec": bv} for i in range(NCORES)]


def kernel(A, prop_state, W, b, _trace=False):
    nc = _get_program()
    in_maps = make_in_maps(prop_state, W, b)
    res = bass_utils.run_bass_kernel_spmd(
        nc, in_maps, core_ids=list(range(NCORES)), trace=_trace)
    out = np.stack(
        [res.results[i]["out"] for i in range(NCORES)], axis=0
    ).astype(np.float32)
    if _trace:
        kernel.last_results = res
    return out


# revision 39
# speedup vs baseline: 1.1205x; 1.0138x over previous
"""Trainium2 Bass kernel for nn_AlignModel.

Computes out[b, j, i] = sigmoid(simp[b,j]·w_s + orig[b,i]·w_o + bias) where
orig/simp are the two halves of prop_state[b] ([B, 2S, D] -> [B,S,D] each),
w_o = W[0,:D], w_s = W[0,D:].

Sharding: data-parallel over batch B=8 across the 8 NeuronCores. Each
core processes its own batch: in x [4096, 512], out [2048, 2048].

Precision strategy (tolerance is rel-err < 2e-2; this kernel measures
~1.3e-3): x and W are downcast to fp16 on the host (dots accumulate in
f32 on-device), the output is stored as fp16 and upcast to f32 on the
host.  Per-core HBM traffic drops from 8 MiB in + 16 MiB out (f32) to
4 MiB in + 8 MiB out.  Measured ~68.8us vs the 95-98us f32 baseline.

Schedule (every choice below is measured-HW-driven):
  - w arrives host-replicated [2, 128, D] (one fast contiguous DMA per
    half; an on-device zero-stride broadcast DMA serialized on a single
    DRAM page at ~40 GB/s), b as a host-built fp16 row.
  - orig is partition-OUTER (i = p*16+n) in chunks of 2-5 n-slices:
    2-5 KiB descriptors run each HWDGE queue ~2x faster than the 1-2
    KiB ones partition-inner forces.  Chunks alternate between the two
    HWDGE queues (Sync + Scalar engines); a single queue saturates at
    ~200-280 GB/s, two together reach ~400+.
  - simp stays partition-inner (the sigmoid bias needs [128,1] columns)
    and is WAW-gated (tiny DVE write emitted BEFORE the dma_start) to
    release mid-phase-1a so it cannot starve the orig stream.
  - All 32 dots are ONE fused DVE scalar_tensor_tensor each (mul +
    free-axis reduce in a single 1x pass, ~670ns); ScalarE runs only
    the 16 output sigmoids (its 2.0us/tile ACTIVATE is the throughput
    wall of the kernel: a 32us stream).
  - s_o row formation at end of phase 1a, off the critical engines:
    PE transpose (tr[n,p] = s_o[p*16+n], f32 PSUM) -> DVE masked
    expansion row_sp[n,(p,ns)] = tr[n,p]*(n==ns) via a zero-stride
    broadcast view reading PSUM directly -> 4 all-ones rank-16 MMs
    accumulate the diagonal onto the b seed in sob PSUM.
  - Sigmoid table preloaded via a dummy activation while ScalarE idles;
    each output row-tile is ONE ScalarE op
      out_t = Sigmoid(sob_psum + bias_col_t)   (PSUM -> SBUF fp16).
  - Stores: tiles 0/15 single (early/late stream edges), pairs on Sync.

Rejected-by-measurement: TensorTensorReduce (crashes HW), partition-
inner fp16 loads (1 KiB desc too slow), ScalarE Copy-accum dots (705ns
accum outpaced by 410ns muls), GpSimd bulk elementwise (~29 Gelem/s),
strided-PSUM matmul outputs (bank-crossing), a PE-outer-product +
DVE-reciprocal path for 4 tiles (its es/eo feed chain and PE rank-1
MM rate land later than ScalarE just finishing the tiles itself).
"""

import numpy as np

import concourse.mybir as mybir
from concourse import bacc, bass_utils
from concourse.masks import make_identity
from concourse.tile import TileContext

P = 128          # partitions
D = 512          # feature dim
S = 2048         # sents
NT = S // P      # 16 tiles per half
OCHUNKS = [2, 5, 5, 4]         # orig n-slices (fp16, desc 2-5KiB)
OQ = ["sync", "scalar", "sync", "scalar"]  # c0 leads the sync queue
SCH = 4          # simp tiles per group (1 MiB)
NSC = NT // SCH
SQ = ["sync", "scalar", "sync", "scalar"]
GATE_AT = {6: 0, 8: 1, 10: 2, 12: 3}   # orig dot t -> simp group released
NCORES = 8
F32 = mybir.dt.float32
F16 = mybir.dt.float16


def _kernel_body(tc, out, x, w, bvec):
    nc = tc.nc
    # orig partition-outer: i = p*NT + n ; simp partition-inner: j = n*P + p
    xo_re = x[0:S, :].rearrange("(p n) d -> p n d", n=NT)
    xs_re = x[S:2 * S, :].rearrange("(n p) d -> p n d", p=P)
    eng = {"sync": nc.sync, "scalar": nc.scalar}

    with (
        tc.tile_pool(name="consts", bufs=1) as cpool,
        tc.tile_pool(name="xin", bufs=1) as xpool,
        tc.tile_pool(name="scratch", bufs=4) as spool,
        tc.tile_pool(name="outbuf", bufs=4) as opool,
        tc.tile_pool(name="psum", bufs=1, space="PSUM") as ppool,
        tc.tile_pool(name="trpsum", bufs=2, space="PSUM") as tpool,
    ):
        w_bc = cpool.tile([P, 2 * D], F16, tag="wbc")
        nc.scalar.dma_start(out=w_bc[:, 0:D], in_=w[0])
        nc.scalar.dma_start(out=w_bc[:, D:2 * D], in_=w[1])
        b_row = cpool.tile([1, 512], F16, tag="brow")
        nc.gpsimd.dma_start(out=b_row, in_=bvec)

        # --- orig input stream: chunks alternate between the two queues ---
        xo_tiles = []
        n0 = 0
        for c, sz in enumerate(OCHUNKS):
            xo = xpool.tile([P, sz, D], F16, tag=f"xo{c}", name=f"xo{c}")
            eng[OQ[c]].dma_start(out=xo, in_=xo_re[:, n0:n0 + sz, :])
            xo_tiles.append(xo)
            n0 += sz

        xs_tiles = [
            xpool.tile([P, SCH, D], F16, tag=f"xs{g}", name=f"xs{g}")
            for g in range(NSC)
        ]

        ones_row = cpool.tile([1, P], F16, tag="ones")
        nc.gpsimd.memset(ones_row, 1.0)
        # preload the Sigmoid table while ScalarE is idle (the implicit
        # ACT_TABLE_LOAD otherwise lands right before the first sigmoid)
        warm = cpool.tile([1, 2], F32, tag="warm")
        nc.gpsimd.memset(warm, 0.0)
        nc.scalar.activation(warm[:, 1:2], warm[:, 0:1],
                             mybir.ActivationFunctionType.Sigmoid)
        ident = cpool.tile([P, P], F32, tag="ident")
        make_identity(nc, ident)
        ones16 = cpool.tile([NT, P], F16, tag="ones16")
        nc.gpsimd.memset(ones16, 1.0)
        # mask16[m, (p, ns)] = (m == ns): the diagonal selector that turns
        # the partition-broadcast of tr16 into the interleaved s_o row.
        mask16 = cpool.tile([NT, S], F16, tag="mask16")
        nc.gpsimd.memset(mask16, 0.0)
        mask3 = mask16.rearrange("m (p ns) -> m p ns", ns=NT)
        nc.gpsimd.affine_select(
            out=mask3, in_=mask3, compare_op=mybir.AluOpType.not_equal,
            fill=1.0, base=0, pattern=[[0, P], [-1, NT]],
            channel_multiplier=1)

        s_o_mat = cpool.tile([P, NT], F32, tag="somat")   # s_o[p*16+n] @ [p,n]
        s_sb_mat = cpool.tile([P, NT], F32, tag="ssmat")  # s_s col t
        row_sp = cpool.tile([NT, S], F16, tag="rowsp")
        sob_psum = ppool.tile([P, S], F32, tag="sob")     # b + s_o, every row

        # bias seeds (FD=512 blocks, matching the accumulation MMs below)
        for jj in range(S // 512):
            nc.tensor.matmul(sob_psum[:, jj * 512:(jj + 1) * 512], ones_row,
                             b_row, start=True, stop=False,
                             skip_group_check=True)

        # --- phase 1a: orig dots ---
        n0 = 0
        for c, sz in enumerate(OCHUNKS):
            xo = xo_tiles[c]
            for blk in range(sz):
                t = n0 + blk
                prod = spool.tile([P, D], F16, tag="prod", name=f"po{t}")
                nc.vector.scalar_tensor_tensor(
                    out=prod, in0=xo[:, blk, :], scalar=1.0,
                    in1=w_bc[:, 0:D],
                    op0=mybir.AluOpType.mult, op1=mybir.AluOpType.mult,
                    accum_out=s_o_mat[:, t:t + 1])
                if t in GATE_AT:
                    # WAW gate: emitted BEFORE the simp dma_start below so
                    # the load waits for this write.
                    g = GATE_AT[t]
                    nc.vector.tensor_copy(
                        out=xs_tiles[g][0:1, 0, 0:1],
                        in_=prod[0:1, 0:1])
            n0 += sz

        # s_o -> sob: transpose, cast, masked diagonal expansion (GpSimd,
        # zero-stride broadcast read), 4 all-ones rank-16 MMs.
        tr = tpool.tile([NT, P], F32, tag="tr", name="trso")
        nc.tensor.transpose(tr, s_o_mat, ident)
        tr_bc = tr.unsqueeze(2).broadcast_to([NT, P, NT])
        nc.vector.tensor_tensor(
            out=row_sp.rearrange("m (p ns) -> m p ns", ns=NT),
            in0=tr_bc, in1=mask3, op=mybir.AluOpType.mult)
        for jj in range(S // 512):
            nc.tensor.matmul(sob_psum[:, jj * 512:(jj + 1) * 512], ones16,
                             row_sp[:, jj * 512:(jj + 1) * 512],
                             start=False, stop=True, skip_group_check=True)

        # simp loads: emitted after the gate writes (WAW order)
        for g in range(NSC):
            eng[SQ[g]].dma_start(out=xs_tiles[g],
                                 in_=xs_re[:, g * SCH:(g + 1) * SCH, :])

        # --- phase 1b + 2: simp dots, then the sigmoid stream ---
        o_sb = None
        for g in range(NSC):
            xs = xs_tiles[g]
            for blk in range(SCH):
                t = g * SCH + blk
                prod = spool.tile([P, D], F16, tag="prod", name=f"ps{t}")
                nc.vector.scalar_tensor_tensor(
                    out=prod, in0=xs[:, blk, :], scalar=1.0,
                    in1=w_bc[:, D:2 * D],
                    op0=mybir.AluOpType.mult, op1=mybir.AluOpType.mult,
                    accum_out=s_sb_mat[:, t:t + 1])
            for blk in range(SCH):
                t = g * SCH + blk
                if t in (0, NT - 1):
                    o_sb = opool.tile([P, 2, S], F16, tag="osb",
                                      name=f"osingle{t}")
                    nc.scalar.activation(
                        o_sb[:, 0, :], sob_psum,
                        mybir.ActivationFunctionType.Sigmoid,
                        bias=s_sb_mat[:, t:t + 1], scale=1.0)
                    nc.sync.dma_start(out=out[t * P:(t + 1) * P, :],
                                      in_=o_sb[:, 0, :])
                    continue
                q = (t - 1) % 2
                if q == 0:
                    o_sb = opool.tile([P, 2, S], F16, tag="osb",
                                      name=f"opair{t // 2}")
                nc.scalar.activation(
                    o_sb[:, q, :], sob_psum,
                    mybir.ActivationFunctionType.Sigmoid,
                    bias=s_sb_mat[:, t:t + 1],
                    scale=1.0,
                )
                if q == 1:
                    r0 = (t - 1) * P
                    dst = out[r0:r0 + 2 * P, :].rearrange(
                        "(q p) i -> p q i", p=P)
                    nc.sync.dma_start(out=dst, in_=o_sb)


def build_program():
    nc = bacc.Bacc(
        "TRN2",
        debug=False,
        target_bir_lowering=False,
        num_devices=NCORES,
    )
    x = nc.dram_tensor("x", [2 * S, D], F16, kind="ExternalInput").ap()
    w = nc.dram_tensor("w", [2, P, D], F16, kind="ExternalInput").ap()
    bvec = nc.dram_tensor("bvec", [1, 512], F16, kind="ExternalInput").ap()
    out = nc.dram_tensor("out", [S, S], F16, kind="ExternalOutput").ap()
    with TileContext(nc) as tc:
        _kernel_body(tc, out, x, w, bvec)
    nc.compile()
    return nc


_PROGRAM = None


def _get_program():
    global _PROGRAM
    if _PROGRAM is None:
        _PROGRAM = build_program()
    return _PROGRAM


def make_in_maps(prop_state, W, b):
    prop = np.ascontiguousarray(np.asarray(prop_state).astype(np.float16))
    w2 = np.asarray(W, dtype=np.float32).reshape(2, 1, D).astype(np.float16)
    w = np.ascontiguousarray(np.broadcast_to(w2, (2, P, D)))
    bv = np.ascontiguousarray(np.broadcast_to(
        np.asarray(b, dtype=np.float16).reshape(1, 1), (1, 512)))
    assert prop.shape == (NCORES, 2 * S, D), prop.shape
    return [{"x": prop[i], "w": w, "bvec": bv} for i in range(NCORES)]


def kernel(A, prop_state, W, b, _trace=False):
    nc = _get_program()
    in_maps = make_in_maps(prop_state, W, b)
    res = bass_utils.run_bass_kernel_spmd(
        nc, in_maps, core_ids=list(range(NCORES)), trace=_trace)
    out = np.stack(
        [res.results[i]["out"] for i in range(NCORES)], axis=0
    ).astype(np.float32)
    if _trace:
        kernel.last_results = res
    return out
